# revision 6
# baseline (speedup 1.0000x reference)
"""Trainium2 Bass kernel for ABMIL-MoE-LoRA linear layer.

Reference computation (B=4, N=2048, D_IN=D_OUT=4096, E=8, R=16, D_ATT=128):
    base = x @ W.T + bias
    v = tanh(x @ V.T); u = sigmoid(x @ U.T)
    rw = sigmoid((v*u) @ router_W.T)                    # [B,N,E]
    lora = x @ A_e  (per expert)                        # [B,N,E,R]
    out = base + sum_e rw[...,e] * (lora_e @ B_e)

Strategy: data-parallel over the B*N = 8192 tokens across 8 NeuronCores
(1024 tokens/core, weights replicated). All matmuls run in bf16 on the
TensorEngine with fp32 PSUM accumulation. Host-side prep pre-transposes
every operand so the contraction dim lands on SBUF partitions.

Schedule: the router/LoRA-down projections are interleaved into the first
two output-column sweeps (k-tile by k-tile, matching DMA arrival order) so
the TensorEngine never starves while x / weights stream in. Those two
sweeps accumulate base-matmul partials into SBUF (PSUM banks are the
scarce resource); later sweeps use the classic 8-bank PSUM accumulation
with the MoE up-projection matmul fused into the same accumulation group.

Self-contained: hardcodes all shapes; only imports installed packages.
"""

import numpy as np
import ml_dtypes

BF16 = ml_dtypes.bfloat16

# Problem shapes (hardcoded per spec)
B, N, D_IN, D_OUT = 4, 2048, 4096, 4096
E, R, D_ATT = 8, 16, 128
TOKENS = B * N            # 8192
N_CORES = 8
T = TOKENS // N_CORES     # 1024 tokens per core
KT = D_IN // 128          # 32 contraction k-tiles
OC = 512                  # output-column chunk per PSUM bank
NOC = D_OUT // OC         # 8 o-chunks
TT = T // 128             # 8 token tiles per core
KH = 2                    # weight streamed in 2 k-halves
KHT = KT // KH            # 16 k-tiles per half

_CACHE = {}


def _get_nc():
    if "nc" in _CACHE:
        return _CACHE["nc"]

    import concourse.tile as tile
    import concourse.mybir as mybir
    from concourse import bacc

    dt = mybir.dt
    AFT = mybir.ActivationFunctionType
    nc = bacc.Bacc("TRN2", target_bir_lowering=False, debug=False)

    xT = nc.declare_dram_parameter("xT", [D_IN, T], dt.bfloat16, isOutput=False)
    wT = nc.declare_dram_parameter("wT", [D_IN, D_OUT], dt.bfloat16, isOutput=False)
    projT = nc.declare_dram_parameter("projT", [D_IN, 384], dt.bfloat16, isOutput=False)
    rwrep = nc.declare_dram_parameter("rwrep", [128, 128], dt.bfloat16, isOutput=False)
    bcat = nc.declare_dram_parameter("bcat", [E * R, D_OUT], dt.bfloat16, isOutput=False)
    biasr = nc.declare_dram_parameter("biasr", [128, D_OUT], dt.bfloat16, isOutput=False)
    out = nc.declare_dram_parameter("out", [T, D_OUT], dt.float32, isOutput=True)

    xT_ap, wT_ap, projT_ap = xT.ap(), wT.ap(), projT.ap()
    rwrep_ap, bcat_ap, biasr_ap, out_ap = rwrep.ap(), bcat.ap(), biasr.ap(), out.ap()

    with tile.TileContext(nc) as tc:
        with (
            tc.tile_pool(name="xpool", bufs=1) as xpool,
            tc.tile_pool(name="wpool", bufs=2) as wpool,
            tc.tile_pool(name="w0pool", bufs=1) as w0pool,
            tc.tile_pool(name="projpool", bufs=6) as projpool,
            tc.tile_pool(name="const", bufs=1) as constp,
            tc.tile_pool(name="inter", bufs=1) as inter,
            tc.tile_pool(name="accpool", bufs=1) as accpool,
            tc.tile_pool(name="opool", bufs=4) as opool,
            tc.tile_pool(name="ps", bufs=8, space="PSUM") as psp,
        ):
            xsb = xpool.tile([128, KT * T], dt.bfloat16, tag="xsb")
            vub = inter.tile([128, T], dt.bfloat16, tag="vub")
            rwb = inter.tile([128, T], dt.float32, tag="rwb")
            wtb = inter.tile([128, T], dt.bfloat16, tag="wtb")
            lsb0 = inter.tile([128, 512], dt.float32, tag="lsb0")
            acc = {}  # (oc, t) -> SBUF fp32 partial-sum tile for sweeps 0/1

            def ps_tile(name):
                return psp.tile([128, 512], dt.float32, tag="ps", name=name)

            # ---- sweeps 0 and 1: router half-sweep h fused with the base
            # matmul for o-chunk 0, token-half h. The oc0 weight chunk stays
            # resident across both sweeps; each (t) runs one full 32-k PSUM
            # accumulation group. Pointwise DMA demand stays under the HBM
            # limit so the TensorEngine never starves while x streams in. ----
            w0sb = w0pool.tile([128, KT * OC], dt.bfloat16, tag="w0sb")
            ocs0 = slice(0, OC)
            rps = {}
            for h, trange in ((0, range(0, 4)), (1, range(4, 8))):
                vps = ps_tile(f"vps{h}")
                ups = ps_tile(f"ups{h}")
                lps = ps_tile(f"lps{h}")
                rps[h] = (vps, ups, lps)
                pst = {t: ps_tile(f"pst0_{t}") for t in trange}
                for k in range(KT):
                    # DMA issue in consumption order
                    nc.sync.dma_start(
                        xsb[:, k * T + h * 512 : k * T + (h + 1) * 512],
                        xT_ap[k * 128 : (k + 1) * 128, h * 512 : (h + 1) * 512],
                    )
                    pt = projpool.tile(
                        [128, 384], dt.bfloat16, tag="proj", name=f"proj{h}_{k}"
                    )
                    nc.sync.dma_start(pt[:], projT_ap[k * 128 : (k + 1) * 128, :])
                    if h == 0:
                        nc.sync.dma_start(
                            w0sb[:, k * OC : (k + 1) * OC],
                            wT_ap[k * 128 : (k + 1) * 128, ocs0],
                        )
                    if h == 0 and k == 0:
                        biassb = constp.tile([128, D_OUT], dt.bfloat16, tag="biassb")
                        nc.sync.dma_start(biassb[:], biasr_ap[:])
                    if h == 1 and k == 0:
                        rwrepsb = constp.tile([128, 128], dt.bfloat16, tag="rwrepsb")
                        nc.sync.dma_start(rwrepsb[:], rwrep_ap[:])
                        bcatsb = constp.tile([128, D_OUT], dt.bfloat16, tag="bcatsb")
                        nc.sync.dma_start(bcatsb[:], bcat_ap[:])

                    st, sp = k == 0, k == KT - 1
                    rx = xsb[:, k * T + h * 512 : k * T + (h + 1) * 512]
                    nc.tensor.matmul(vps[:], pt[:, 0:128], rx, start=st, stop=sp)
                    nc.tensor.matmul(ups[:], pt[:, 128:256], rx, start=st, stop=sp)
                    nc.tensor.matmul(lps[:], pt[:, 256:384], rx, start=st, stop=sp)
                    for t in trange:
                        nc.tensor.matmul(
                            pst[t][:],
                            xsb[:, k * T + t * 128 : k * T + (t + 1) * 128],
                            w0sb[:, k * OC : (k + 1) * OC],
                            start=st,
                            stop=sp,
                        )
                for t in trange:
                    a = accpool.tile(
                        [128, 512], dt.float32, tag=f"acc0_{t}", name=f"acc0_{t}"
                    )
                    acc[(0, t)] = a
                    nc.vector.tensor_add(a[:], pst[t][:], biassb[:, ocs0])

                # router epilogue for half h: free the 3 PSUM accumulators
                vtmp = inter.tile([128, 512], dt.float32, tag="vtmp", name=f"vtmp{h}")
                utmp = inter.tile([128, 512], dt.float32, tag="utmp", name=f"utmp{h}")
                nc.scalar.activation(vtmp[:], vps[:], AFT.Tanh)
                nc.scalar.activation(utmp[:], ups[:], AFT.Sigmoid)
                nc.vector.tensor_mul(vub[:, h * 512 : (h + 1) * 512], vtmp[:], utmp[:])
                if h == 0:
                    nc.vector.tensor_copy(lsb0[:], lps[:])

            # ---- scores + gates + weighted lora (wtb) ----
            lps1 = rps[1][2]
            for h in range(2):
                sl = slice(h * 512, (h + 1) * 512)
                sps = ps_tile(f"sps{h}")
                nc.tensor.matmul(sps[:], rwrepsb[:], vub[:, sl], start=True, stop=True)
                nc.scalar.activation(rwb[:, sl], sps[:], AFT.Sigmoid)
            nc.vector.tensor_mul(wtb[:, 0:512], lsb0[:], rwb[:, 0:512])
            nc.vector.tensor_mul(wtb[:, 512:1024], lps1[:], rwb[:, 512:1024])

            # ---- sweeps 2..7: classic 8-bank PSUM accumulation.
            # The MoE finish matmuls for sweeps 0/1 are interleaved into
            # spare PSUM slots of sweeps 2/3 so they never serialize on
            # DVE bank recycling. ----
            extras = []
            for oc in (0,):
                ocs_f = slice(oc * OC, (oc + 1) * OC)
                for t in range(TT):
                    def emit_finish(oc=oc, t=t, ocs_f=ocs_f):
                        pm = ps_tile(f"pm{oc}_{t}")
                        nc.tensor.matmul(
                            pm[:],
                            wtb[:, t * 128 : (t + 1) * 128],
                            bcatsb[:, ocs_f],
                            start=True,
                            stop=True,
                        )
                        osb = opool.tile(
                            [128, 512], dt.float32, tag="osb", name=f"osbf{oc}_{t}"
                        )
                        nc.vector.tensor_add(osb[:], pm[:], acc[(oc, t)][:])
                        nc.sync.dma_start(
                            out_ap[t * 128 : (t + 1) * 128, ocs_f], osb[:]
                        )
                    extras.append(emit_finish)
            extras_it = iter(extras)

            def classic_sweep(oc, take_extras_kh0, take_extras_kh1):
                ocs = slice(oc * OC, (oc + 1) * OC)
                pst = [None] * TT
                for kh in range(KH):
                    wsb = wpool.tile(
                        [128, KHT * OC], dt.bfloat16, tag="wsb", name=f"wsb{oc}_{kh}"
                    )
                    for kk in range(KHT):
                        k = kh * KHT + kk
                        nc.sync.dma_start(
                            wsb[:, kk * OC : (kk + 1) * OC],
                            wT_ap[k * 128 : (k + 1) * 128, ocs],
                        )
                    for t in range(TT):
                        if kh == 0:
                            pst[t] = ps_tile(f"pst{oc}_{t}")
                        for kk in range(KHT):
                            k = kh * KHT + kk
                            nc.tensor.matmul(
                                pst[t][:],
                                xsb[:, k * T + t * 128 : k * T + (t + 1) * 128],
                                wsb[:, kk * OC : (kk + 1) * OC],
                                start=(k == 0),
                                stop=False,
                            )
                        if kh == KH - 1:
                            nc.tensor.matmul(
                                pst[t][:],
                                wtb[:, t * 128 : (t + 1) * 128],
                                bcatsb[:, ocs],
                                start=False,
                                stop=True,
                            )
                            osb = opool.tile([128, 512], dt.float32, tag="osb")
                            nc.vector.tensor_add(osb[:], pst[t][:], biassb[:, ocs])
                            nc.sync.dma_start(
                                out_ap[t * 128 : (t + 1) * 128, ocs], osb[:]
                            )
                        # interleave a pending finish matmul when a spare
                        # PSUM bank exists (kh0: pst[0..t]+pm <= 8)
                        take = (take_extras_kh1 if kh == KH - 1
                                else (take_extras_kh0 and 1 <= t <= 6))
                        if take:
                            fn = next(extras_it, None)
                            if fn is not None:
                                fn()

            classic_sweep(1, False, True)
            for oc in range(2, NOC):
                classic_sweep(oc, True, True)
            assert next(extras_it, None) is None

    nc.compile()
    _CACHE["nc"] = nc
    return nc


def _prep_in_maps(x, weight, bias, router_V, router_U, router_W, experts_A, experts_B):
    xT_all = np.ascontiguousarray(
        x.reshape(TOKENS, D_IN).T.astype(BF16)
    )  # [D_IN, TOKENS]
    wT = np.ascontiguousarray(weight.T.astype(BF16))  # [D_IN, D_OUT]
    projT = np.concatenate(
        [
            router_V.T,  # [D_IN, 128]
            router_U.T,  # [D_IN, 128]
            experts_A.transpose(1, 0, 2).reshape(D_IN, E * R),  # [D_IN, 128]
        ],
        axis=1,
    ).astype(BF16)
    rwrep = np.ascontiguousarray(np.repeat(router_W, R, axis=0).T.astype(BF16))
    bcat = np.ascontiguousarray(experts_B.reshape(E * R, D_OUT).astype(BF16))
    biasr = np.ascontiguousarray(
        np.broadcast_to(bias.astype(BF16), (128, D_OUT))
    )

    in_maps = []
    for c in range(N_CORES):
        in_maps.append(
            {
                "xT": np.ascontiguousarray(xT_all[:, c * T : (c + 1) * T]),
                "wT": wT,
                "projT": projT,
                "rwrep": rwrep,
                "bcat": bcat,
                "biasr": biasr,
            }
        )
    return in_maps


def _gather(results):
    out = np.concatenate(
        [np.asarray(results[c]["out"], dtype=np.float32) for c in range(N_CORES)],
        axis=0,
    )
    return out.reshape(B, N, D_OUT)


def kernel(x, weight, bias, router_V, router_U, router_W, experts_A, experts_B):
    from concourse.bass_utils import run_bass_kernel_spmd

    nc = _get_nc()
    in_maps = _prep_in_maps(
        x, weight, bias, router_V, router_U, router_W, experts_A, experts_B
    )
    res = run_bass_kernel_spmd(nc, in_maps, list(range(N_CORES)))
    return _gather(res.results)


def run_traced(x, weight, bias, router_V, router_U, router_W, experts_A, experts_B):
    """Correctness + HW timing run (profiled). Returns (out, exec_time_ns, trace)."""
    import concourse.bass_utils as bass_utils

    bass_utils.upload_artifacts = lambda tmpdir: tmpdir  # no fileshare here
    nc = _get_nc()
    in_maps = _prep_in_maps(
        x, weight, bias, router_V, router_U, router_W, experts_A, experts_B
    )
    res = bass_utils.run_bass_kernel_spmd(
        nc, in_maps, list(range(N_CORES)), trace=True
    )
    trace_path = None
    if res.instructions_and_trace is not None:
        trace_path = res.instructions_and_trace[1]
    return _gather(res.results), res.exec_time_ns, trace_path


# revision 7
# speedup vs baseline: 1.0204x; 1.0204x over previous
"""Trainium2 Bass kernel for ABMIL-MoE-LoRA linear layer.

Reference computation (B=4, N=2048, D_IN=D_OUT=4096, E=8, R=16, D_ATT=128):
    base = x @ W.T + bias
    v = tanh(x @ V.T); u = sigmoid(x @ U.T)
    rw = sigmoid((v*u) @ router_W.T)                    # [B,N,E]
    lora = x @ A_e  (per expert)                        # [B,N,E,R]
    out = base + sum_e rw[...,e] * (lora_e @ B_e)

Strategy: data-parallel over the B*N = 8192 tokens across 8 NeuronCores
(1024 tokens/core, weights replicated). All matmuls run in bf16 on the
TensorEngine with fp32 PSUM accumulation. Host-side prep pre-transposes
every operand so the contraction dim lands on SBUF partitions.

Schedule: the router/LoRA-down projections are interleaved into the first
two output-column sweeps (k-tile by k-tile, matching DMA arrival order) so
the TensorEngine never starves while x / weights stream in. Those two
sweeps accumulate base-matmul partials into SBUF (PSUM banks are the
scarce resource); later sweeps use the classic 8-bank PSUM accumulation
with the MoE up-projection matmul fused into the same accumulation group.

Self-contained: hardcodes all shapes; only imports installed packages.
"""

import numpy as np
import ml_dtypes

BF16 = ml_dtypes.bfloat16

# Problem shapes (hardcoded per spec)
B, N, D_IN, D_OUT = 4, 2048, 4096, 4096
E, R, D_ATT = 8, 16, 128
TOKENS = B * N            # 8192
N_CORES = 8
T = TOKENS // N_CORES     # 1024 tokens per core
KT = D_IN // 128          # 32 contraction k-tiles
OC = 512                  # output-column chunk per PSUM bank
NOC = D_OUT // OC         # 8 o-chunks
TT = T // 128             # 8 token tiles per core
KH = 2                    # weight streamed in 2 k-halves
KHT = KT // KH            # 16 k-tiles per half

_CACHE = {}


def _get_nc():
    if "nc" in _CACHE:
        return _CACHE["nc"]

    import concourse.tile as tile
    import concourse.mybir as mybir
    from concourse import bacc

    dt = mybir.dt
    AFT = mybir.ActivationFunctionType
    nc = bacc.Bacc("TRN2", target_bir_lowering=False, debug=False)

    xT = nc.declare_dram_parameter("xT", [D_IN, T], dt.bfloat16, isOutput=False)
    wT = nc.declare_dram_parameter("wT", [D_IN, D_OUT], dt.bfloat16, isOutput=False)
    projT = nc.declare_dram_parameter("projT", [D_IN, 384], dt.bfloat16, isOutput=False)
    rwrep = nc.declare_dram_parameter("rwrep", [128, 128], dt.bfloat16, isOutput=False)
    bcat = nc.declare_dram_parameter("bcat", [E * R, D_OUT], dt.bfloat16, isOutput=False)
    biasr = nc.declare_dram_parameter("biasr", [128, D_OUT], dt.bfloat16, isOutput=False)
    out = nc.declare_dram_parameter("out", [T, D_OUT], dt.float32, isOutput=True)

    xT_ap, wT_ap, projT_ap = xT.ap(), wT.ap(), projT.ap()
    rwrep_ap, bcat_ap, biasr_ap, out_ap = rwrep.ap(), bcat.ap(), biasr.ap(), out.ap()

    with tile.TileContext(nc) as tc:
        with (
            tc.tile_pool(name="xpool", bufs=1) as xpool,
            tc.tile_pool(name="wpool", bufs=2) as wpool,
            tc.tile_pool(name="w0pool", bufs=1) as w0pool,
            tc.tile_pool(name="const", bufs=1) as constp,
            tc.tile_pool(name="inter", bufs=1) as inter,
            tc.tile_pool(name="accpool", bufs=1) as accpool,
            tc.tile_pool(name="opool", bufs=4) as opool,
            tc.tile_pool(name="ps", bufs=8, space="PSUM") as psp,
        ):
            xsb = xpool.tile([128, KT * T], dt.bfloat16, tag="xsb")
            vub = inter.tile([128, T], dt.bfloat16, tag="vub")
            rwb = inter.tile([128, T], dt.float32, tag="rwb")
            wtb = inter.tile([128, T], dt.bfloat16, tag="wtb")
            lsb0 = inter.tile([128, 512], dt.float32, tag="lsb0")
            acc = {}  # (oc, t) -> SBUF fp32 partial-sum tile for sweeps 0/1

            def ps_tile(name):
                return psp.tile([128, 512], dt.float32, tag="ps", name=name)

            # ---- sweeps 0 and 1: router half-sweep h fused with the base
            # matmul for o-chunk 0, token-half h. The oc0 weight chunk stays
            # resident across both sweeps; each (t) runs one full 32-k PSUM
            # accumulation group. Pointwise DMA demand stays under the HBM
            # limit so the TensorEngine never starves while x streams in. ----
            w0sb = w0pool.tile([128, KT * OC], dt.bfloat16, tag="w0sb")
            projsb = w0pool.tile([128, KT * 384], dt.bfloat16, tag="projsb")
            ocs0 = slice(0, OC)
            rps = {}
            for h, trange in ((0, range(0, 4)), (1, range(4, 8))):
                # all DMAs for this sweep upfront, in consumption order, so
                # the HW DGE queues stay deep and sustain full HBM bandwidth
                for k in range(KT):
                    nc.sync.dma_start(
                        xsb[:, k * T + h * 512 : k * T + (h + 1) * 512],
                        xT_ap[k * 128 : (k + 1) * 128, h * 512 : (h + 1) * 512],
                    )
                    if h == 0:
                        nc.sync.dma_start(
                            projsb[:, k * 384 : (k + 1) * 384],
                            projT_ap[k * 128 : (k + 1) * 128, :],
                        )
                        nc.sync.dma_start(
                            w0sb[:, k * OC : (k + 1) * OC],
                            wT_ap[k * 128 : (k + 1) * 128, ocs0],
                        )
                if h == 0:
                    biassb = constp.tile([128, D_OUT], dt.bfloat16, tag="biassb")
                    nc.sync.dma_start(biassb[:], biasr_ap[:])
                else:
                    rwrepsb = constp.tile([128, 128], dt.bfloat16, tag="rwrepsb")
                    nc.sync.dma_start(rwrepsb[:], rwrep_ap[:])
                    bcatsb = constp.tile([128, D_OUT], dt.bfloat16, tag="bcatsb")
                    nc.sync.dma_start(bcatsb[:], bcat_ap[:])

                vps = ps_tile(f"vps{h}")
                ups = ps_tile(f"ups{h}")
                lps = ps_tile(f"lps{h}")
                rps[h] = (vps, ups, lps)
                pst = {t: ps_tile(f"pst0_{t}") for t in trange}
                for k in range(KT):
                    st, sp = k == 0, k == KT - 1
                    rx = xsb[:, k * T + h * 512 : k * T + (h + 1) * 512]
                    pj = projsb[:, k * 384 : (k + 1) * 384]
                    nc.tensor.matmul(vps[:], pj[:, 0:128], rx, start=st, stop=sp)
                    nc.tensor.matmul(ups[:], pj[:, 128:256], rx, start=st, stop=sp)
                    nc.tensor.matmul(lps[:], pj[:, 256:384], rx, start=st, stop=sp)
                    for t in trange:
                        nc.tensor.matmul(
                            pst[t][:],
                            xsb[:, k * T + t * 128 : k * T + (t + 1) * 128],
                            w0sb[:, k * OC : (k + 1) * OC],
                            start=st,
                            stop=sp,
                        )
                for t in trange:
                    a = accpool.tile(
                        [128, 512], dt.float32, tag=f"acc0_{t}", name=f"acc0_{t}"
                    )
                    acc[(0, t)] = a
                    nc.vector.tensor_add(a[:], pst[t][:], biassb[:, ocs0])

                # router epilogue for half h: free the 3 PSUM accumulators
                vtmp = inter.tile([128, 512], dt.float32, tag="vtmp", name=f"vtmp{h}")
                utmp = inter.tile([128, 512], dt.float32, tag="utmp", name=f"utmp{h}")
                nc.scalar.activation(vtmp[:], vps[:], AFT.Tanh)
                nc.scalar.activation(utmp[:], ups[:], AFT.Sigmoid)
                nc.vector.tensor_mul(vub[:, h * 512 : (h + 1) * 512], vtmp[:], utmp[:])
                if h == 0:
                    nc.vector.tensor_copy(lsb0[:], lps[:])

            # ---- scores + gates + weighted lora (wtb) ----
            lps1 = rps[1][2]
            for h in range(2):
                sl = slice(h * 512, (h + 1) * 512)
                sps = ps_tile(f"sps{h}")
                nc.tensor.matmul(sps[:], rwrepsb[:], vub[:, sl], start=True, stop=True)
                nc.scalar.activation(rwb[:, sl], sps[:], AFT.Sigmoid)
            nc.vector.tensor_mul(wtb[:, 0:512], lsb0[:], rwb[:, 0:512])
            nc.vector.tensor_mul(wtb[:, 512:1024], lps1[:], rwb[:, 512:1024])

            # ---- sweeps 2..7: classic 8-bank PSUM accumulation.
            # The MoE finish matmuls for sweeps 0/1 are interleaved into
            # spare PSUM slots of sweeps 2/3 so they never serialize on
            # DVE bank recycling. ----
            extras = []
            for oc in (0,):
                ocs_f = slice(oc * OC, (oc + 1) * OC)
                for t in range(TT):
                    def emit_finish(oc=oc, t=t, ocs_f=ocs_f):
                        pm = ps_tile(f"pm{oc}_{t}")
                        nc.tensor.matmul(
                            pm[:],
                            wtb[:, t * 128 : (t + 1) * 128],
                            bcatsb[:, ocs_f],
                            start=True,
                            stop=True,
                        )
                        osb = opool.tile(
                            [128, 512], dt.float32, tag="osb", name=f"osbf{oc}_{t}"
                        )
                        nc.vector.tensor_add(osb[:], pm[:], acc[(oc, t)][:])
                        nc.sync.dma_start(
                            out_ap[t * 128 : (t + 1) * 128, ocs_f], osb[:]
                        )
                    extras.append(emit_finish)
            extras_it = iter(extras)

            def classic_sweep(oc, take_extras_kh0, take_extras_kh1):
                ocs = slice(oc * OC, (oc + 1) * OC)
                pst = [None] * TT
                for kh in range(KH):
                    wsb = wpool.tile(
                        [128, KHT * OC], dt.bfloat16, tag="wsb", name=f"wsb{oc}_{kh}"
                    )
                    for kk in range(KHT):
                        k = kh * KHT + kk
                        nc.sync.dma_start(
                            wsb[:, kk * OC : (kk + 1) * OC],
                            wT_ap[k * 128 : (k + 1) * 128, ocs],
                        )
                    for t in range(TT):
                        if kh == 0:
                            pst[t] = ps_tile(f"pst{oc}_{t}")
                        for kk in range(KHT):
                            k = kh * KHT + kk
                            nc.tensor.matmul(
                                pst[t][:],
                                xsb[:, k * T + t * 128 : k * T + (t + 1) * 128],
                                wsb[:, kk * OC : (kk + 1) * OC],
                                start=(k == 0),
                                stop=False,
                            )
                        if kh == KH - 1:
                            nc.tensor.matmul(
                                pst[t][:],
                                wtb[:, t * 128 : (t + 1) * 128],
                                bcatsb[:, ocs],
                                start=False,
                                stop=True,
                            )
                            osb = opool.tile([128, 512], dt.float32, tag="osb")
                            nc.vector.tensor_add(osb[:], pst[t][:], biassb[:, ocs])
                            nc.sync.dma_start(
                                out_ap[t * 128 : (t + 1) * 128, ocs], osb[:]
                            )
                        # interleave a pending finish matmul when a spare
                        # PSUM bank exists (kh0: pst[0..t]+pm <= 8)
                        take = (take_extras_kh1 if kh == KH - 1
                                else (take_extras_kh0 and 1 <= t <= 6))
                        if take:
                            fn = next(extras_it, None)
                            if fn is not None:
                                fn()

            classic_sweep(1, False, True)
            for oc in range(2, NOC):
                classic_sweep(oc, True, True)
            assert next(extras_it, None) is None

    nc.compile()
    _CACHE["nc"] = nc
    return nc


def _prep_in_maps(x, weight, bias, router_V, router_U, router_W, experts_A, experts_B):
    xT_all = np.ascontiguousarray(
        x.reshape(TOKENS, D_IN).T.astype(BF16)
    )  # [D_IN, TOKENS]
    wT = np.ascontiguousarray(weight.T.astype(BF16))  # [D_IN, D_OUT]
    projT = np.concatenate(
        [
            router_V.T,  # [D_IN, 128]
            router_U.T,  # [D_IN, 128]
            experts_A.transpose(1, 0, 2).reshape(D_IN, E * R),  # [D_IN, 128]
        ],
        axis=1,
    ).astype(BF16)
    rwrep = np.ascontiguousarray(np.repeat(router_W, R, axis=0).T.astype(BF16))
    bcat = np.ascontiguousarray(experts_B.reshape(E * R, D_OUT).astype(BF16))
    biasr = np.ascontiguousarray(
        np.broadcast_to(bias.astype(BF16), (128, D_OUT))
    )

    in_maps = []
    for c in range(N_CORES):
        in_maps.append(
            {
                "xT": np.ascontiguousarray(xT_all[:, c * T : (c + 1) * T]),
                "wT": wT,
                "projT": projT,
                "rwrep": rwrep,
                "bcat": bcat,
                "biasr": biasr,
            }
        )
    return in_maps


def _gather(results):
    out = np.concatenate(
        [np.asarray(results[c]["out"], dtype=np.float32) for c in range(N_CORES)],
        axis=0,
    )
    return out.reshape(B, N, D_OUT)


def kernel(x, weight, bias, router_V, router_U, router_W, experts_A, experts_B):
    from concourse.bass_utils import run_bass_kernel_spmd

    nc = _get_nc()
    in_maps = _prep_in_maps(
        x, weight, bias, router_V, router_U, router_W, experts_A, experts_B
    )
    res = run_bass_kernel_spmd(nc, in_maps, list(range(N_CORES)))
    return _gather(res.results)


def run_traced(x, weight, bias, router_V, router_U, router_W, experts_A, experts_B):
    """Correctness + HW timing run (profiled). Returns (out, exec_time_ns, trace)."""
    import concourse.bass_utils as bass_utils

    bass_utils.upload_artifacts = lambda tmpdir: tmpdir  # no fileshare here
    nc = _get_nc()
    in_maps = _prep_in_maps(
        x, weight, bias, router_V, router_U, router_W, experts_A, experts_B
    )
    res = bass_utils.run_bass_kernel_spmd(
        nc, in_maps, list(range(N_CORES)), trace=True
    )
    trace_path = None
    if res.instructions_and_trace is not None:
        trace_path = res.instructions_and_trace[1]
    return _gather(res.results), res.exec_time_ns, trace_path


# revision 8
# speedup vs baseline: 1.0519x; 1.0309x over previous
"""Trainium2 Bass kernel for ABMIL-MoE-LoRA linear layer.

Reference computation (B=4, N=2048, D_IN=D_OUT=4096, E=8, R=16, D_ATT=128):
    base = x @ W.T + bias
    v = tanh(x @ V.T); u = sigmoid(x @ U.T)
    rw = sigmoid((v*u) @ router_W.T)                    # [B,N,E]
    lora = x @ A_e  (per expert)                        # [B,N,E,R]
    out = base + sum_e rw[...,e] * (lora_e @ B_e)

Strategy: data-parallel over the B*N = 8192 tokens across 8 NeuronCores
(1024 tokens/core, weights replicated). All matmuls run in bf16 on the
TensorEngine with fp32 PSUM accumulation. Host-side prep pre-transposes
every operand so the contraction dim lands on SBUF partitions.

Schedule: the router/LoRA-down projections are interleaved into the first
two output-column sweeps (k-tile by k-tile, matching DMA arrival order) so
the TensorEngine never starves while x / weights stream in. Those two
sweeps accumulate base-matmul partials into SBUF (PSUM banks are the
scarce resource); later sweeps use the classic 8-bank PSUM accumulation
with the MoE up-projection matmul fused into the same accumulation group.

Self-contained: hardcodes all shapes; only imports installed packages.
"""

import numpy as np
import ml_dtypes

BF16 = ml_dtypes.bfloat16

# Problem shapes (hardcoded per spec)
B, N, D_IN, D_OUT = 4, 2048, 4096, 4096
E, R, D_ATT = 8, 16, 128
TOKENS = B * N            # 8192
N_CORES = 8
T = TOKENS // N_CORES     # 1024 tokens per core
KT = D_IN // 128          # 32 contraction k-tiles
OC = 512                  # output-column chunk per PSUM bank
NOC = D_OUT // OC         # 8 o-chunks
TT = T // 128             # 8 token tiles per core
KH = 2                    # weight streamed in 2 k-halves
KHT = KT // KH            # 16 k-tiles per half

_CACHE = {}


def _get_nc():
    if "nc" in _CACHE:
        return _CACHE["nc"]

    import concourse.tile as tile
    import concourse.mybir as mybir
    from concourse import bacc

    dt = mybir.dt
    AFT = mybir.ActivationFunctionType
    nc = bacc.Bacc("TRN2", target_bir_lowering=False, debug=False)

    xT = nc.declare_dram_parameter("xT", [D_IN, T], dt.bfloat16, isOutput=False)
    wT = nc.declare_dram_parameter("wT", [D_IN, D_OUT], dt.bfloat16, isOutput=False)
    projT = nc.declare_dram_parameter("projT", [D_IN, 384], dt.bfloat16, isOutput=False)
    rwrep = nc.declare_dram_parameter("rwrep", [128, 128], dt.bfloat16, isOutput=False)
    bcat = nc.declare_dram_parameter("bcat", [E * R, D_OUT], dt.bfloat16, isOutput=False)
    biasr = nc.declare_dram_parameter("biasr", [128, D_OUT], dt.bfloat16, isOutput=False)
    out = nc.declare_dram_parameter("out", [T, D_OUT], dt.float32, isOutput=True)

    xT_ap, wT_ap, projT_ap = xT.ap(), wT.ap(), projT.ap()
    rwrep_ap, bcat_ap, biasr_ap, out_ap = rwrep.ap(), bcat.ap(), biasr.ap(), out.ap()

    with tile.TileContext(nc) as tc:
        with (
            tc.tile_pool(name="xpool", bufs=1) as xpool,
            tc.tile_pool(name="wpool", bufs=2) as wpool,
            tc.tile_pool(name="w0pool", bufs=1) as w0pool,
            tc.tile_pool(name="const", bufs=1) as constp,
            tc.tile_pool(name="inter", bufs=1) as inter,
            tc.tile_pool(name="accpool", bufs=1) as accpool,
            tc.tile_pool(name="opool", bufs=4) as opool,
            tc.tile_pool(name="ps", bufs=8, space="PSUM") as psp,
        ):
            xsb = xpool.tile([128, KT * T], dt.bfloat16, tag="xsb")
            vub = inter.tile([128, T], dt.bfloat16, tag="vub")
            rwb = inter.tile([128, T], dt.float32, tag="rwb")
            wtb = inter.tile([128, T], dt.bfloat16, tag="wtb")
            lsb0 = inter.tile([128, 512], dt.float32, tag="lsb0")
            acc = {}  # (oc, t) -> SBUF fp32 partial-sum tile for sweeps 0/1

            def ps_tile(name):
                return psp.tile([128, 512], dt.float32, tag="ps", name=name)

            # ---- sweeps 0 and 1: router half-sweep h fused with the base
            # matmul for o-chunk 0, token-half h. The oc0 weight chunk stays
            # resident across both sweeps; each (t) runs one full 32-k PSUM
            # accumulation group. Pointwise DMA demand stays under the HBM
            # limit so the TensorEngine never starves while x streams in. ----
            w0sb = w0pool.tile([128, KT * OC], dt.bfloat16, tag="w0sb")
            projsb = w0pool.tile([128, KT * 384], dt.bfloat16, tag="projsb")
            xT_r = xT_ap.rearrange("(a p) t -> p a t", p=128)
            wT_r = wT_ap.rearrange("(a p) o -> p a o", p=128)
            projT_r = projT_ap.rearrange("(a p) c -> p a c", p=128)
            xsb_r = xsb.rearrange("p (a t) -> p a t", a=KT)
            w0sb_r = w0sb.rearrange("p (a o) -> p a o", a=KT)
            projsb_r = projsb.rearrange("p (a c) -> p a c", a=KT)
            ocs0 = slice(0, OC)
            rps = {}
            for h, trange in ((0, range(0, 4)), (1, range(4, 8))):
                # all DMAs for this sweep upfront, in consumption order and
                # batched 4 k-tiles per transfer (~0.6us sync-engine issue
                # cost per DMA caps bandwidth at ~1.6 x size GB/us)
                for k0 in range(0, KT, 4):
                    ka = slice(k0, k0 + 4)
                    nc.sync.dma_start(
                        xsb_r[:, ka, h * 512 : (h + 1) * 512],
                        xT_r[:, ka, h * 512 : (h + 1) * 512],
                    )
                    if h == 0:
                        nc.sync.dma_start(projsb_r[:, ka, :], projT_r[:, ka, :])
                        nc.sync.dma_start(w0sb_r[:, ka, :], wT_r[:, ka, ocs0])
                if h == 0:
                    biassb = constp.tile([128, D_OUT], dt.bfloat16, tag="biassb")
                    nc.sync.dma_start(biassb[:], biasr_ap[:])
                else:
                    rwrepsb = constp.tile([128, 128], dt.bfloat16, tag="rwrepsb")
                    nc.sync.dma_start(rwrepsb[:], rwrep_ap[:])
                    bcatsb = constp.tile([128, D_OUT], dt.bfloat16, tag="bcatsb")
                    nc.sync.dma_start(bcatsb[:], bcat_ap[:])

                vps = ps_tile(f"vps{h}")
                ups = ps_tile(f"ups{h}")
                lps = ps_tile(f"lps{h}")
                rps[h] = (vps, ups, lps)
                pst = {t: ps_tile(f"pst0_{t}") for t in trange}
                for k in range(KT):
                    st, sp = k == 0, k == KT - 1
                    rx = xsb[:, k * T + h * 512 : k * T + (h + 1) * 512]
                    pj = projsb[:, k * 384 : (k + 1) * 384]
                    nc.tensor.matmul(vps[:], pj[:, 0:128], rx, start=st, stop=sp)
                    nc.tensor.matmul(ups[:], pj[:, 128:256], rx, start=st, stop=sp)
                    nc.tensor.matmul(lps[:], pj[:, 256:384], rx, start=st, stop=sp)
                    for t in trange:
                        nc.tensor.matmul(
                            pst[t][:],
                            xsb[:, k * T + t * 128 : k * T + (t + 1) * 128],
                            w0sb[:, k * OC : (k + 1) * OC],
                            start=st,
                            stop=sp,
                        )
                for t in trange:
                    a = accpool.tile(
                        [128, 512], dt.float32, tag=f"acc0_{t}", name=f"acc0_{t}"
                    )
                    acc[(0, t)] = a
                    nc.vector.tensor_add(a[:], pst[t][:], biassb[:, ocs0])

                # router epilogue for half h: free the 3 PSUM accumulators
                vtmp = inter.tile([128, 512], dt.float32, tag="vtmp", name=f"vtmp{h}")
                utmp = inter.tile([128, 512], dt.float32, tag="utmp", name=f"utmp{h}")
                nc.scalar.activation(vtmp[:], vps[:], AFT.Tanh)
                nc.scalar.activation(utmp[:], ups[:], AFT.Sigmoid)
                nc.vector.tensor_mul(vub[:, h * 512 : (h + 1) * 512], vtmp[:], utmp[:])
                if h == 0:
                    nc.vector.tensor_copy(lsb0[:], lps[:])

            # ---- scores + gates + weighted lora (wtb) ----
            lps1 = rps[1][2]
            for h in range(2):
                sl = slice(h * 512, (h + 1) * 512)
                sps = ps_tile(f"sps{h}")
                nc.tensor.matmul(sps[:], rwrepsb[:], vub[:, sl], start=True, stop=True)
                nc.scalar.activation(rwb[:, sl], sps[:], AFT.Sigmoid)
            nc.vector.tensor_mul(wtb[:, 0:512], lsb0[:], rwb[:, 0:512])
            nc.vector.tensor_mul(wtb[:, 512:1024], lps1[:], rwb[:, 512:1024])

            # ---- sweeps 2..7: classic 8-bank PSUM accumulation.
            # The MoE finish matmuls for sweeps 0/1 are interleaved into
            # spare PSUM slots of sweeps 2/3 so they never serialize on
            # DVE bank recycling. ----
            extras = []
            for oc in (0,):
                ocs_f = slice(oc * OC, (oc + 1) * OC)
                for t in range(TT):
                    def emit_finish(oc=oc, t=t, ocs_f=ocs_f):
                        pm = ps_tile(f"pm{oc}_{t}")
                        nc.tensor.matmul(
                            pm[:],
                            wtb[:, t * 128 : (t + 1) * 128],
                            bcatsb[:, ocs_f],
                            start=True,
                            stop=True,
                        )
                        osb = opool.tile(
                            [128, 512], dt.float32, tag="osb", name=f"osbf{oc}_{t}"
                        )
                        nc.vector.tensor_add(osb[:], pm[:], acc[(oc, t)][:])
                        nc.sync.dma_start(
                            out_ap[t * 128 : (t + 1) * 128, ocs_f], osb[:]
                        )
                    extras.append(emit_finish)
            extras_it = iter(extras)

            def classic_sweep(oc, take_extras_kh0, take_extras_kh1):
                ocs = slice(oc * OC, (oc + 1) * OC)
                pst = [None] * TT
                for kh in range(KH):
                    wsb = wpool.tile(
                        [128, KHT * OC], dt.bfloat16, tag="wsb", name=f"wsb{oc}_{kh}"
                    )
                    wsb_r = wsb.rearrange("p (a o) -> p a o", a=KHT)
                    for kk0 in range(0, KHT, 4):
                        nc.sync.dma_start(
                            wsb_r[:, kk0 : kk0 + 4, :],
                            wT_r[:, kh * KHT + kk0 : kh * KHT + kk0 + 4, ocs],
                        )
                    for t in range(TT):
                        if kh == 0:
                            pst[t] = ps_tile(f"pst{oc}_{t}")
                        for kk in range(KHT):
                            k = kh * KHT + kk
                            nc.tensor.matmul(
                                pst[t][:],
                                xsb[:, k * T + t * 128 : k * T + (t + 1) * 128],
                                wsb[:, kk * OC : (kk + 1) * OC],
                                start=(k == 0),
                                stop=False,
                            )
                        if kh == KH - 1:
                            nc.tensor.matmul(
                                pst[t][:],
                                wtb[:, t * 128 : (t + 1) * 128],
                                bcatsb[:, ocs],
                                start=False,
                                stop=True,
                            )
                            osb = opool.tile([128, 512], dt.float32, tag="osb")
                            nc.vector.tensor_add(osb[:], pst[t][:], biassb[:, ocs])
                            nc.sync.dma_start(
                                out_ap[t * 128 : (t + 1) * 128, ocs], osb[:]
                            )
                        # interleave a pending finish matmul when a spare
                        # PSUM bank exists (kh0: pst[0..t]+pm <= 8)
                        take = (take_extras_kh1 if kh == KH - 1
                                else (take_extras_kh0 and 1 <= t <= 6))
                        if take:
                            fn = next(extras_it, None)
                            if fn is not None:
                                fn()

            classic_sweep(1, False, True)
            for oc in range(2, NOC):
                classic_sweep(oc, True, True)
            assert next(extras_it, None) is None

    nc.compile()
    _CACHE["nc"] = nc
    return nc


def _prep_in_maps(x, weight, bias, router_V, router_U, router_W, experts_A, experts_B):
    xT_all = np.ascontiguousarray(
        x.reshape(TOKENS, D_IN).T.astype(BF16)
    )  # [D_IN, TOKENS]
    wT = np.ascontiguousarray(weight.T.astype(BF16))  # [D_IN, D_OUT]
    projT = np.concatenate(
        [
            router_V.T,  # [D_IN, 128]
            router_U.T,  # [D_IN, 128]
            experts_A.transpose(1, 0, 2).reshape(D_IN, E * R),  # [D_IN, 128]
        ],
        axis=1,
    ).astype(BF16)
    rwrep = np.ascontiguousarray(np.repeat(router_W, R, axis=0).T.astype(BF16))
    bcat = np.ascontiguousarray(experts_B.reshape(E * R, D_OUT).astype(BF16))
    biasr = np.ascontiguousarray(
        np.broadcast_to(bias.astype(BF16), (128, D_OUT))
    )

    in_maps = []
    for c in range(N_CORES):
        in_maps.append(
            {
                "xT": np.ascontiguousarray(xT_all[:, c * T : (c + 1) * T]),
                "wT": wT,
                "projT": projT,
                "rwrep": rwrep,
                "bcat": bcat,
                "biasr": biasr,
            }
        )
    return in_maps


def _gather(results):
    out = np.concatenate(
        [np.asarray(results[c]["out"], dtype=np.float32) for c in range(N_CORES)],
        axis=0,
    )
    return out.reshape(B, N, D_OUT)


def kernel(x, weight, bias, router_V, router_U, router_W, experts_A, experts_B):
    from concourse.bass_utils import run_bass_kernel_spmd

    nc = _get_nc()
    in_maps = _prep_in_maps(
        x, weight, bias, router_V, router_U, router_W, experts_A, experts_B
    )
    res = run_bass_kernel_spmd(nc, in_maps, list(range(N_CORES)))
    return _gather(res.results)


def run_traced(x, weight, bias, router_V, router_U, router_W, experts_A, experts_B):
    """Correctness + HW timing run (profiled). Returns (out, exec_time_ns, trace)."""
    import concourse.bass_utils as bass_utils

    bass_utils.upload_artifacts = lambda tmpdir: tmpdir  # no fileshare here
    nc = _get_nc()
    in_maps = _prep_in_maps(
        x, weight, bias, router_V, router_U, router_W, experts_A, experts_B
    )
    res = bass_utils.run_bass_kernel_spmd(
        nc, in_maps, list(range(N_CORES)), trace=True
    )
    trace_path = None
    if res.instructions_and_trace is not None:
        trace_path = res.instructions_and_trace[1]
    return _gather(res.results), res.exec_time_ns, trace_path


# revision 9
# speedup vs baseline: 1.0560x; 1.0038x over previous
"""Trainium2 Bass kernel for ABMIL-MoE-LoRA linear layer.

Reference computation (B=4, N=2048, D_IN=D_OUT=4096, E=8, R=16, D_ATT=128):
    base = x @ W.T + bias
    v = tanh(x @ V.T); u = sigmoid(x @ U.T)
    rw = sigmoid((v*u) @ router_W.T)                    # [B,N,E]
    lora = x @ A_e  (per expert)                        # [B,N,E,R]
    out = base + sum_e rw[...,e] * (lora_e @ B_e)

Strategy: data-parallel over the B*N = 8192 tokens across 8 NeuronCores
(1024 tokens/core, weights replicated). All matmuls run in bf16 on the
TensorEngine with fp32 PSUM accumulation. Host-side prep pre-transposes
every operand so the contraction dim lands on SBUF partitions.

Schedule: the router/LoRA-down projections are interleaved into the first
two output-column sweeps (k-tile by k-tile, matching DMA arrival order) so
the TensorEngine never starves while x / weights stream in. Those two
sweeps accumulate base-matmul partials into SBUF (PSUM banks are the
scarce resource); later sweeps use the classic 8-bank PSUM accumulation
with the MoE up-projection matmul fused into the same accumulation group.

Self-contained: hardcodes all shapes; only imports installed packages.
"""

import numpy as np
import ml_dtypes

BF16 = ml_dtypes.bfloat16

# Problem shapes (hardcoded per spec)
B, N, D_IN, D_OUT = 4, 2048, 4096, 4096
E, R, D_ATT = 8, 16, 128
TOKENS = B * N            # 8192
N_CORES = 8
T = TOKENS // N_CORES     # 1024 tokens per core
KT = D_IN // 128          # 32 contraction k-tiles
OC = 512                  # output-column chunk per PSUM bank
NOC = D_OUT // OC         # 8 o-chunks
TT = T // 128             # 8 token tiles per core
KH = 2                    # weight streamed in 2 k-halves
KHT = KT // KH            # 16 k-tiles per half

_CACHE = {}


def _get_nc():
    if "nc" in _CACHE:
        return _CACHE["nc"]

    import concourse.tile as tile
    import concourse.mybir as mybir
    from concourse import bacc

    dt = mybir.dt
    AFT = mybir.ActivationFunctionType
    nc = bacc.Bacc("TRN2", target_bir_lowering=False, debug=False)

    xT = nc.declare_dram_parameter("xT", [D_IN, T], dt.bfloat16, isOutput=False)
    wT = nc.declare_dram_parameter("wT", [D_IN, D_OUT], dt.bfloat16, isOutput=False)
    projT = nc.declare_dram_parameter("projT", [D_IN, 384], dt.bfloat16, isOutput=False)
    rwrep = nc.declare_dram_parameter("rwrep", [128, 128], dt.bfloat16, isOutput=False)
    bcat = nc.declare_dram_parameter("bcat", [E * R, D_OUT], dt.bfloat16, isOutput=False)
    biasr = nc.declare_dram_parameter("biasr", [128, D_OUT], dt.bfloat16, isOutput=False)
    out = nc.declare_dram_parameter("out", [T, D_OUT], dt.float32, isOutput=True)

    xT_ap, wT_ap, projT_ap = xT.ap(), wT.ap(), projT.ap()
    rwrep_ap, bcat_ap, biasr_ap, out_ap = rwrep.ap(), bcat.ap(), biasr.ap(), out.ap()

    with tile.TileContext(nc) as tc:
        with (
            tc.tile_pool(name="xpool", bufs=1) as xpool,
            tc.tile_pool(name="wpool", bufs=2) as wpool,
            tc.tile_pool(name="w0pool", bufs=1) as w0pool,
            tc.tile_pool(name="const", bufs=1) as constp,
            tc.tile_pool(name="inter", bufs=1) as inter,
            tc.tile_pool(name="accpool", bufs=1) as accpool,
            tc.tile_pool(name="opool", bufs=4) as opool,
            tc.tile_pool(name="ps", bufs=8, space="PSUM") as psp,
        ):
            xsb = xpool.tile([128, KT * T], dt.bfloat16, tag="xsb")
            vub = inter.tile([128, T], dt.bfloat16, tag="vub")
            rwb = inter.tile([128, T], dt.float32, tag="rwb")
            wtb = inter.tile([128, T], dt.bfloat16, tag="wtb")
            lsb0 = inter.tile([128, 512], dt.float32, tag="lsb0")
            acc = {}  # (oc, t) -> SBUF fp32 partial-sum tile for sweeps 0/1

            def ps_tile(name):
                return psp.tile([128, 512], dt.float32, tag="ps", name=name)

            # ---- sweeps 0 and 1: router half-sweep h fused with the base
            # matmul for o-chunk 0, token-half h. The oc0 weight chunk stays
            # resident across both sweeps; each (t) runs one full 32-k PSUM
            # accumulation group. Pointwise DMA demand stays under the HBM
            # limit so the TensorEngine never starves while x streams in. ----
            w0sb = w0pool.tile([128, KT * OC], dt.bfloat16, tag="w0sb")
            projsb = w0pool.tile([128, KT * 384], dt.bfloat16, tag="projsb")
            xT_r = xT_ap.rearrange("(a p) t -> p a t", p=128)
            wT_r = wT_ap.rearrange("(a p) o -> p a o", p=128)
            projT_r = projT_ap.rearrange("(a p) c -> p a c", p=128)
            xsb_r = xsb.rearrange("p (a t) -> p a t", a=KT)
            w0sb_r = w0sb.rearrange("p (a o) -> p a o", a=KT)
            projsb_r = projsb.rearrange("p (a c) -> p a c", a=KT)
            ocs0 = slice(0, OC)
            rps = {}
            for h, trange in ((0, range(0, 4)), (1, range(4, 8))):
                # all DMAs for this sweep upfront, in consumption order and
                # batched 4 k-tiles per transfer (~0.6us sync-engine issue
                # cost per DMA caps bandwidth at ~1.6 x size GB/us)
                # first 8 k-tiles as single-tile DMAs (low first-transfer
                # latency so the PE starts early), then 4-tile batches
                batches = [slice(k, k + 1) for k in range(8)] + [
                    slice(k0, k0 + 4) for k0 in range(8, KT, 4)
                ] if h == 0 else [slice(k0, k0 + 4) for k0 in range(0, KT, 4)]
                for ka in batches:
                    nc.sync.dma_start(
                        xsb_r[:, ka, h * 512 : (h + 1) * 512],
                        xT_r[:, ka, h * 512 : (h + 1) * 512],
                    )
                    if h == 0:
                        nc.sync.dma_start(projsb_r[:, ka, :], projT_r[:, ka, :])
                        nc.sync.dma_start(w0sb_r[:, ka, :], wT_r[:, ka, ocs0])
                if h == 0:
                    biassb = constp.tile([128, D_OUT], dt.bfloat16, tag="biassb")
                    nc.sync.dma_start(biassb[:], biasr_ap[:])
                else:
                    rwrepsb = constp.tile([128, 128], dt.bfloat16, tag="rwrepsb")
                    nc.sync.dma_start(rwrepsb[:], rwrep_ap[:])
                    bcatsb = constp.tile([128, D_OUT], dt.bfloat16, tag="bcatsb")
                    nc.sync.dma_start(bcatsb[:], bcat_ap[:])

                vps = ps_tile(f"vps{h}")
                ups = ps_tile(f"ups{h}")
                lps = ps_tile(f"lps{h}")
                rps[h] = (vps, ups, lps)
                pst = {t: ps_tile(f"pst0_{t}") for t in trange}
                DELAY = 6 if h == 1 else 0

                def base_mms(k, trange=trange, pst=pst):
                    for t in trange:
                        nc.tensor.matmul(
                            pst[t][:],
                            xsb[:, k * T + t * 128 : k * T + (t + 1) * 128],
                            w0sb[:, k * OC : (k + 1) * OC],
                            start=(k == 0),
                            stop=(k == KT - 1),
                        )

                for k in range(KT):
                    st, sp = k == 0, k == KT - 1
                    rx = xsb[:, k * T + h * 512 : k * T + (h + 1) * 512]
                    pj = projsb[:, k * 384 : (k + 1) * 384]
                    nc.tensor.matmul(vps[:], pj[:, 0:128], rx, start=st, stop=sp)
                    nc.tensor.matmul(ups[:], pj[:, 128:256], rx, start=st, stop=sp)
                    nc.tensor.matmul(lps[:], pj[:, 256:384], rx, start=st, stop=sp)
                    if k >= DELAY:
                        base_mms(k - DELAY)
                for k in range(KT - DELAY, KT):
                    base_mms(k)
                for t in trange:
                    a = accpool.tile(
                        [128, 512], dt.float32, tag=f"acc0_{t}", name=f"acc0_{t}"
                    )
                    acc[(0, t)] = a
                    nc.vector.tensor_add(a[:], pst[t][:], biassb[:, ocs0])

                # router epilogue for half h: free the 3 PSUM accumulators
                vtmp = inter.tile([128, 512], dt.float32, tag="vtmp", name=f"vtmp{h}")
                utmp = inter.tile([128, 512], dt.float32, tag="utmp", name=f"utmp{h}")
                nc.scalar.activation(vtmp[:], vps[:], AFT.Tanh)
                nc.scalar.activation(utmp[:], ups[:], AFT.Sigmoid)
                nc.vector.tensor_mul(vub[:, h * 512 : (h + 1) * 512], vtmp[:], utmp[:])
                if h == 0:
                    nc.vector.tensor_copy(lsb0[:], lps[:])

            # ---- scores + gates + weighted lora (wtb) ----
            lps1 = rps[1][2]
            for h in range(2):
                sl = slice(h * 512, (h + 1) * 512)
                sps = ps_tile(f"sps{h}")
                nc.tensor.matmul(sps[:], rwrepsb[:], vub[:, sl], start=True, stop=True)
                nc.scalar.activation(rwb[:, sl], sps[:], AFT.Sigmoid)
            nc.vector.tensor_mul(wtb[:, 0:512], lsb0[:], rwb[:, 0:512])
            nc.vector.tensor_mul(wtb[:, 512:1024], lps1[:], rwb[:, 512:1024])

            # ---- sweeps 2..7: classic 8-bank PSUM accumulation.
            # The MoE finish matmuls for sweeps 0/1 are interleaved into
            # spare PSUM slots of sweeps 2/3 so they never serialize on
            # DVE bank recycling. ----
            extras = []
            for oc in (0,):
                ocs_f = slice(oc * OC, (oc + 1) * OC)
                for t in range(TT):
                    def emit_finish(oc=oc, t=t, ocs_f=ocs_f):
                        pm = ps_tile(f"pm{oc}_{t}")
                        nc.tensor.matmul(
                            pm[:],
                            wtb[:, t * 128 : (t + 1) * 128],
                            bcatsb[:, ocs_f],
                            start=True,
                            stop=True,
                        )
                        osb = opool.tile(
                            [128, 512], dt.float32, tag="osb", name=f"osbf{oc}_{t}"
                        )
                        nc.vector.tensor_add(osb[:], pm[:], acc[(oc, t)][:])
                        nc.sync.dma_start(
                            out_ap[t * 128 : (t + 1) * 128, ocs_f], osb[:]
                        )
                    extras.append(emit_finish)
            extras_it = iter(extras)

            def classic_sweep(oc, take_extras_kh0, take_extras_kh1):
                ocs = slice(oc * OC, (oc + 1) * OC)
                pst = [None] * TT
                for kh in range(KH):
                    wsb = wpool.tile(
                        [128, KHT * OC], dt.bfloat16, tag="wsb", name=f"wsb{oc}_{kh}"
                    )
                    wsb_r = wsb.rearrange("p (a o) -> p a o", a=KHT)
                    for kk0 in range(0, KHT, 4):
                        nc.sync.dma_start(
                            wsb_r[:, kk0 : kk0 + 4, :],
                            wT_r[:, kh * KHT + kk0 : kh * KHT + kk0 + 4, ocs],
                        )
                    for t in range(TT):
                        if kh == 0:
                            pst[t] = ps_tile(f"pst{oc}_{t}")
                        for kk in range(KHT):
                            k = kh * KHT + kk
                            nc.tensor.matmul(
                                pst[t][:],
                                xsb[:, k * T + t * 128 : k * T + (t + 1) * 128],
                                wsb[:, kk * OC : (kk + 1) * OC],
                                start=(k == 0),
                                stop=False,
                            )
                        if kh == KH - 1:
                            nc.tensor.matmul(
                                pst[t][:],
                                wtb[:, t * 128 : (t + 1) * 128],
                                bcatsb[:, ocs],
                                start=False,
                                stop=True,
                            )
                            osb = opool.tile([128, 512], dt.float32, tag="osb")
                            nc.vector.tensor_add(osb[:], pst[t][:], biassb[:, ocs])
                            nc.sync.dma_start(
                                out_ap[t * 128 : (t + 1) * 128, ocs], osb[:]
                            )
                        # interleave a pending finish matmul when a spare
                        # PSUM bank exists (kh0: pst[0..t]+pm <= 8)
                        take = (take_extras_kh1 if kh == KH - 1
                                else (take_extras_kh0 and 1 <= t <= 6))
                        if take:
                            fn = next(extras_it, None)
                            if fn is not None:
                                fn()

            classic_sweep(1, False, True)
            for oc in range(2, NOC):
                classic_sweep(oc, True, True)
            assert next(extras_it, None) is None

    nc.compile()
    _CACHE["nc"] = nc
    return nc


def _prep_in_maps(x, weight, bias, router_V, router_U, router_W, experts_A, experts_B):
    xT_all = np.ascontiguousarray(
        x.reshape(TOKENS, D_IN).T.astype(BF16)
    )  # [D_IN, TOKENS]
    wT = np.ascontiguousarray(weight.T.astype(BF16))  # [D_IN, D_OUT]
    projT = np.concatenate(
        [
            router_V.T,  # [D_IN, 128]
            router_U.T,  # [D_IN, 128]
            experts_A.transpose(1, 0, 2).reshape(D_IN, E * R),  # [D_IN, 128]
        ],
        axis=1,
    ).astype(BF16)
    rwrep = np.ascontiguousarray(np.repeat(router_W, R, axis=0).T.astype(BF16))
    bcat = np.ascontiguousarray(experts_B.reshape(E * R, D_OUT).astype(BF16))
    biasr = np.ascontiguousarray(
        np.broadcast_to(bias.astype(BF16), (128, D_OUT))
    )

    in_maps = []
    for c in range(N_CORES):
        in_maps.append(
            {
                "xT": np.ascontiguousarray(xT_all[:, c * T : (c + 1) * T]),
                "wT": wT,
                "projT": projT,
                "rwrep": rwrep,
                "bcat": bcat,
                "biasr": biasr,
            }
        )
    return in_maps


def _gather(results):
    out = np.concatenate(
        [np.asarray(results[c]["out"], dtype=np.float32) for c in range(N_CORES)],
        axis=0,
    )
    return out.reshape(B, N, D_OUT)


def kernel(x, weight, bias, router_V, router_U, router_W, experts_A, experts_B):
    from concourse.bass_utils import run_bass_kernel_spmd

    nc = _get_nc()
    in_maps = _prep_in_maps(
        x, weight, bias, router_V, router_U, router_W, experts_A, experts_B
    )
    res = run_bass_kernel_spmd(nc, in_maps, list(range(N_CORES)))
    return _gather(res.results)


def run_traced(x, weight, bias, router_V, router_U, router_W, experts_A, experts_B):
    """Correctness + HW timing run (profiled). Returns (out, exec_time_ns, trace)."""
    import concourse.bass_utils as bass_utils

    bass_utils.upload_artifacts = lambda tmpdir: tmpdir  # no fileshare here
    nc = _get_nc()
    in_maps = _prep_in_maps(
        x, weight, bias, router_V, router_U, router_W, experts_A, experts_B
    )
    res = bass_utils.run_bass_kernel_spmd(
        nc, in_maps, list(range(N_CORES)), trace=True
    )
    trace_path = None
    if res.instructions_and_trace is not None:
        trace_path = res.instructions_and_trace[1]
    return _gather(res.results), res.exec_time_ns, trace_path


# revision 10
# speedup vs baseline: 1.0586x; 1.0025x over previous
"""Trainium2 Bass kernel for ABMIL-MoE-LoRA linear layer.

Reference computation (B=4, N=2048, D_IN=D_OUT=4096, E=8, R=16, D_ATT=128):
    base = x @ W.T + bias
    v = tanh(x @ V.T); u = sigmoid(x @ U.T)
    rw = sigmoid((v*u) @ router_W.T)                    # [B,N,E]
    lora = x @ A_e  (per expert)                        # [B,N,E,R]
    out = base + sum_e rw[...,e] * (lora_e @ B_e)

Strategy: data-parallel over the B*N = 8192 tokens across 8 NeuronCores
(1024 tokens/core, weights replicated). All matmuls run in bf16 on the
TensorEngine with fp32 PSUM accumulation. Host-side prep pre-transposes
every operand so the contraction dim lands on SBUF partitions.

Schedule: the router/LoRA-down projections are interleaved into the first
two output-column sweeps (k-tile by k-tile, matching DMA arrival order) so
the TensorEngine never starves while x / weights stream in. Those two
sweeps accumulate base-matmul partials into SBUF (PSUM banks are the
scarce resource); later sweeps use the classic 8-bank PSUM accumulation
with the MoE up-projection matmul fused into the same accumulation group.

Self-contained: hardcodes all shapes; only imports installed packages.
"""

import numpy as np
import ml_dtypes

BF16 = ml_dtypes.bfloat16

# Problem shapes (hardcoded per spec)
B, N, D_IN, D_OUT = 4, 2048, 4096, 4096
E, R, D_ATT = 8, 16, 128
TOKENS = B * N            # 8192
N_CORES = 8
T = TOKENS // N_CORES     # 1024 tokens per core
KT = D_IN // 128          # 32 contraction k-tiles
OC = 512                  # output-column chunk per PSUM bank
NOC = D_OUT // OC         # 8 o-chunks
TT = T // 128             # 8 token tiles per core
KH = 2                    # weight streamed in 2 k-halves
KHT = KT // KH            # 16 k-tiles per half

_CACHE = {}


def _get_nc():
    if "nc" in _CACHE:
        return _CACHE["nc"]

    import concourse.tile as tile
    import concourse.mybir as mybir
    from concourse import bacc

    dt = mybir.dt
    AFT = mybir.ActivationFunctionType
    nc = bacc.Bacc("TRN2", target_bir_lowering=False, debug=False)

    xT = nc.declare_dram_parameter("xT", [D_IN, T], dt.bfloat16, isOutput=False)
    wT = nc.declare_dram_parameter("wT", [D_IN, D_OUT], dt.bfloat16, isOutput=False)
    projT = nc.declare_dram_parameter("projT", [D_IN, 384], dt.bfloat16, isOutput=False)
    rwrep = nc.declare_dram_parameter("rwrep", [128, 128], dt.bfloat16, isOutput=False)
    bcat = nc.declare_dram_parameter("bcat", [E * R, D_OUT], dt.bfloat16, isOutput=False)
    biasr = nc.declare_dram_parameter("biasr", [128, D_OUT], dt.bfloat16, isOutput=False)
    out = nc.declare_dram_parameter("out", [T, D_OUT], dt.float32, isOutput=True)

    xT_ap, wT_ap, projT_ap = xT.ap(), wT.ap(), projT.ap()
    rwrep_ap, bcat_ap, biasr_ap, out_ap = rwrep.ap(), bcat.ap(), biasr.ap(), out.ap()

    with tile.TileContext(nc) as tc:
        with (
            tc.tile_pool(name="xpool", bufs=1) as xpool,
            tc.tile_pool(name="wpool", bufs=2) as wpool,
            tc.tile_pool(name="w0pool", bufs=1) as w0pool,
            tc.tile_pool(name="const", bufs=1) as constp,
            tc.tile_pool(name="inter", bufs=1) as inter,
            tc.tile_pool(name="accpool", bufs=1) as accpool,
            tc.tile_pool(name="opool", bufs=4) as opool,
            tc.tile_pool(name="ps", bufs=8, space="PSUM") as psp,
        ):
            xsb = xpool.tile([128, KT * T], dt.bfloat16, tag="xsb")
            vub = inter.tile([128, T], dt.bfloat16, tag="vub")
            rwb = inter.tile([128, T], dt.float32, tag="rwb")
            wtb = inter.tile([128, T], dt.bfloat16, tag="wtb")
            lsb0 = inter.tile([128, 512], dt.float32, tag="lsb0")
            acc = {}  # (oc, t) -> SBUF fp32 partial-sum tile for sweeps 0/1

            def ps_tile(name):
                return psp.tile([128, 512], dt.float32, tag="ps", name=name)

            # ---- sweeps 0 and 1: router half-sweep h fused with the base
            # matmul for o-chunk 0, token-half h. The oc0 weight chunk stays
            # resident across both sweeps; each (t) runs one full 32-k PSUM
            # accumulation group. Pointwise DMA demand stays under the HBM
            # limit so the TensorEngine never starves while x streams in. ----
            w0sb = w0pool.tile([128, KT * OC], dt.bfloat16, tag="w0sb")
            projsb = w0pool.tile([128, KT * 384], dt.bfloat16, tag="projsb")
            xT_r = xT_ap.rearrange("(a p) t -> p a t", p=128)
            wT_r = wT_ap.rearrange("(a p) o -> p a o", p=128)
            projT_r = projT_ap.rearrange("(a p) c -> p a c", p=128)
            xsb_r = xsb.rearrange("p (a t) -> p a t", a=KT)
            w0sb_r = w0sb.rearrange("p (a o) -> p a o", a=KT)
            projsb_r = projsb.rearrange("p (a c) -> p a c", a=KT)
            ocs0 = slice(0, OC)
            rps = {}
            for h, trange in ((0, range(0, 4)), (1, range(4, 8))):
                # all DMAs for this sweep upfront, in consumption order and
                # batched 4 k-tiles per transfer (~0.6us sync-engine issue
                # cost per DMA caps bandwidth at ~1.6 x size GB/us)
                # first 8 k-tiles as single-tile DMAs (low first-transfer
                # latency so the PE starts early), then 4-tile batches
                batches = (
                    [slice(k, k + 1) for k in range(2)]
                    + [slice(k, k + 2) for k in range(2, 8, 2)]
                    + [slice(k0, k0 + 4) for k0 in range(8, KT, 4)]
                ) if h == 0 else [slice(k0, k0 + 4) for k0 in range(0, KT, 4)]
                for ka in batches:
                    nc.sync.dma_start(
                        xsb_r[:, ka, h * 512 : (h + 1) * 512],
                        xT_r[:, ka, h * 512 : (h + 1) * 512],
                    )
                    if h == 0:
                        nc.sync.dma_start(projsb_r[:, ka, :], projT_r[:, ka, :])
                        nc.sync.dma_start(w0sb_r[:, ka, :], wT_r[:, ka, ocs0])
                if h == 0:
                    biassb = constp.tile([128, D_OUT], dt.bfloat16, tag="biassb")
                    nc.sync.dma_start(biassb[:], biasr_ap[:])
                else:
                    rwrepsb = constp.tile([128, 128], dt.bfloat16, tag="rwrepsb")
                    nc.sync.dma_start(rwrepsb[:], rwrep_ap[:])
                    bcatsb = constp.tile([128, D_OUT], dt.bfloat16, tag="bcatsb")
                    nc.sync.dma_start(bcatsb[:], bcat_ap[:])

                vps = ps_tile(f"vps{h}")
                ups = ps_tile(f"ups{h}")
                lps = ps_tile(f"lps{h}")
                rps[h] = (vps, ups, lps)
                pst = {t: ps_tile(f"pst0_{t}") for t in trange}
                DELAY = 6 if h == 1 else 0

                def base_mms(k, trange=trange, pst=pst):
                    for t in trange:
                        nc.tensor.matmul(
                            pst[t][:],
                            xsb[:, k * T + t * 128 : k * T + (t + 1) * 128],
                            w0sb[:, k * OC : (k + 1) * OC],
                            start=(k == 0),
                            stop=(k == KT - 1),
                        )

                for k in range(KT):
                    st, sp = k == 0, k == KT - 1
                    rx = xsb[:, k * T + h * 512 : k * T + (h + 1) * 512]
                    pj = projsb[:, k * 384 : (k + 1) * 384]
                    nc.tensor.matmul(vps[:], pj[:, 0:128], rx, start=st, stop=sp)
                    nc.tensor.matmul(ups[:], pj[:, 128:256], rx, start=st, stop=sp)
                    nc.tensor.matmul(lps[:], pj[:, 256:384], rx, start=st, stop=sp)
                    if k >= DELAY:
                        base_mms(k - DELAY)
                for k in range(KT - DELAY, KT):
                    base_mms(k)
                for t in trange:
                    a = accpool.tile(
                        [128, 512], dt.float32, tag=f"acc0_{t}", name=f"acc0_{t}"
                    )
                    acc[(0, t)] = a
                    nc.vector.tensor_add(a[:], pst[t][:], biassb[:, ocs0])

                # router epilogue for half h: free the 3 PSUM accumulators
                vtmp = inter.tile([128, 512], dt.float32, tag="vtmp", name=f"vtmp{h}")
                utmp = inter.tile([128, 512], dt.float32, tag="utmp", name=f"utmp{h}")
                nc.scalar.activation(vtmp[:], vps[:], AFT.Tanh)
                nc.scalar.activation(utmp[:], ups[:], AFT.Sigmoid)
                nc.vector.tensor_mul(vub[:, h * 512 : (h + 1) * 512], vtmp[:], utmp[:])
                if h == 0:
                    nc.vector.tensor_copy(lsb0[:], lps[:])

            # ---- scores + gates + weighted lora (wtb) ----
            lps1 = rps[1][2]
            for h in range(2):
                sl = slice(h * 512, (h + 1) * 512)
                sps = ps_tile(f"sps{h}")
                nc.tensor.matmul(sps[:], rwrepsb[:], vub[:, sl], start=True, stop=True)
                nc.scalar.activation(rwb[:, sl], sps[:], AFT.Sigmoid)
            nc.vector.tensor_mul(wtb[:, 0:512], lsb0[:], rwb[:, 0:512])
            nc.vector.tensor_mul(wtb[:, 512:1024], lps1[:], rwb[:, 512:1024])

            # ---- sweeps 2..7: classic 8-bank PSUM accumulation.
            # The MoE finish matmuls for sweeps 0/1 are interleaved into
            # spare PSUM slots of sweeps 2/3 so they never serialize on
            # DVE bank recycling. ----
            extras = []
            for oc in (0,):
                ocs_f = slice(oc * OC, (oc + 1) * OC)
                for t in range(TT):
                    def emit_finish(oc=oc, t=t, ocs_f=ocs_f):
                        pm = ps_tile(f"pm{oc}_{t}")
                        nc.tensor.matmul(
                            pm[:],
                            wtb[:, t * 128 : (t + 1) * 128],
                            bcatsb[:, ocs_f],
                            start=True,
                            stop=True,
                        )
                        osb = opool.tile(
                            [128, 512], dt.float32, tag="osb", name=f"osbf{oc}_{t}"
                        )
                        nc.vector.tensor_add(osb[:], pm[:], acc[(oc, t)][:])
                        nc.sync.dma_start(
                            out_ap[t * 128 : (t + 1) * 128, ocs_f], osb[:]
                        )
                    extras.append(emit_finish)
            extras_it = iter(extras)

            def classic_sweep(oc, take_extras_kh0, take_extras_kh1):
                ocs = slice(oc * OC, (oc + 1) * OC)
                pst = [None] * TT
                for kh in range(KH):
                    wsb = wpool.tile(
                        [128, KHT * OC], dt.bfloat16, tag="wsb", name=f"wsb{oc}_{kh}"
                    )
                    wsb_r = wsb.rearrange("p (a o) -> p a o", a=KHT)
                    for kk0 in range(0, KHT, 4):
                        nc.sync.dma_start(
                            wsb_r[:, kk0 : kk0 + 4, :],
                            wT_r[:, kh * KHT + kk0 : kh * KHT + kk0 + 4, ocs],
                        )
                    for t in range(TT):
                        if kh == 0:
                            pst[t] = ps_tile(f"pst{oc}_{t}")
                        for kk in range(KHT):
                            k = kh * KHT + kk
                            nc.tensor.matmul(
                                pst[t][:],
                                xsb[:, k * T + t * 128 : k * T + (t + 1) * 128],
                                wsb[:, kk * OC : (kk + 1) * OC],
                                start=(k == 0),
                                stop=False,
                            )
                        if kh == KH - 1:
                            nc.tensor.matmul(
                                pst[t][:],
                                wtb[:, t * 128 : (t + 1) * 128],
                                bcatsb[:, ocs],
                                start=False,
                                stop=True,
                            )
                            osb = opool.tile([128, 512], dt.float32, tag="osb")
                            nc.vector.tensor_add(osb[:], pst[t][:], biassb[:, ocs])
                            nc.sync.dma_start(
                                out_ap[t * 128 : (t + 1) * 128, ocs], osb[:]
                            )
                        # interleave a pending finish matmul when a spare
                        # PSUM bank exists (kh0: pst[0..t]+pm <= 8)
                        take = (take_extras_kh1 if kh == KH - 1
                                else (take_extras_kh0 and 1 <= t <= 6))
                        if take:
                            fn = next(extras_it, None)
                            if fn is not None:
                                fn()

            classic_sweep(1, False, True)
            for oc in range(2, NOC):
                classic_sweep(oc, True, True)
            assert next(extras_it, None) is None

    nc.compile()
    _CACHE["nc"] = nc
    return nc


def _prep_in_maps(x, weight, bias, router_V, router_U, router_W, experts_A, experts_B):
    xT_all = np.ascontiguousarray(
        x.reshape(TOKENS, D_IN).T.astype(BF16)
    )  # [D_IN, TOKENS]
    wT = np.ascontiguousarray(weight.T.astype(BF16))  # [D_IN, D_OUT]
    projT = np.concatenate(
        [
            router_V.T,  # [D_IN, 128]
            router_U.T,  # [D_IN, 128]
            experts_A.transpose(1, 0, 2).reshape(D_IN, E * R),  # [D_IN, 128]
        ],
        axis=1,
    ).astype(BF16)
    rwrep = np.ascontiguousarray(np.repeat(router_W, R, axis=0).T.astype(BF16))
    bcat = np.ascontiguousarray(experts_B.reshape(E * R, D_OUT).astype(BF16))
    biasr = np.ascontiguousarray(
        np.broadcast_to(bias.astype(BF16), (128, D_OUT))
    )

    in_maps = []
    for c in range(N_CORES):
        in_maps.append(
            {
                "xT": np.ascontiguousarray(xT_all[:, c * T : (c + 1) * T]),
                "wT": wT,
                "projT": projT,
                "rwrep": rwrep,
                "bcat": bcat,
                "biasr": biasr,
            }
        )
    return in_maps


def _gather(results):
    out = np.concatenate(
        [np.asarray(results[c]["out"], dtype=np.float32) for c in range(N_CORES)],
        axis=0,
    )
    return out.reshape(B, N, D_OUT)


def kernel(x, weight, bias, router_V, router_U, router_W, experts_A, experts_B):
    from concourse.bass_utils import run_bass_kernel_spmd

    nc = _get_nc()
    in_maps = _prep_in_maps(
        x, weight, bias, router_V, router_U, router_W, experts_A, experts_B
    )
    res = run_bass_kernel_spmd(nc, in_maps, list(range(N_CORES)))
    return _gather(res.results)


def run_traced(x, weight, bias, router_V, router_U, router_W, experts_A, experts_B):
    """Correctness + HW timing run (profiled). Returns (out, exec_time_ns, trace)."""
    import concourse.bass_utils as bass_utils

    bass_utils.upload_artifacts = lambda tmpdir: tmpdir  # no fileshare here
    nc = _get_nc()
    in_maps = _prep_in_maps(
        x, weight, bias, router_V, router_U, router_W, experts_A, experts_B
    )
    res = bass_utils.run_bass_kernel_spmd(
        nc, in_maps, list(range(N_CORES)), trace=True
    )
    trace_path = None
    if res.instructions_and_trace is not None:
        trace_path = res.instructions_and_trace[1]
    return _gather(res.results), res.exec_time_ns, trace_path
